# revision 17
# baseline (speedup 1.0000x reference)
"""Trainium2 Bass kernel for nn_LowpassDetector.

Computes: power = re^2 + im^2, 5-tap FIR (b), order-4 IIR recurrence (a)
along time, for signal [2, T=16384, B=2048] -> y [T, B].

Strategy: the FIR+IIR cascade is LTI with all poles at radius <= 0.758,
so the combined impulse response decays below 1e-15 within 128 taps.
The whole filter is therefore exactly (to fp32) a block-Toeplitz matmul:
  y_blk[b] = T0 @ x_blk[b] + T1 @ x_blk[b-1]     (b >= 1)
  y_blk[0] = L0 @ x_blk[0]
with L0 the exact 128x128 operator of the reference recurrence
(including its "first 5 samples pass through" initial condition), built
on the host in float64. Channels (2048) are sharded 256 per core across
8 cores; time blocks of 128 map to the TensorEngine contraction dim.

v6 design (from v5's ~91-98 us; trace-driven):
- All I/O fp16 (fp8-e3m4/e4m3 input was simulated on host and fails the
  2e-2 max-rel budget at 3-6.6e-2: the metric is tail-dominated and
  fp8's coarse ulp at large |x| survives the filter). 25.3 MB/core at
  the measured ~420 GB/s sustained = ~60 us hard DMA floor.
- v5's trace: ~7 us fixed preamble, 16 serial input-DMA issues at
  ~0.6 us each, a 23 us tail at 1/3 rate where stores trailed per-SB
  compute, and ~8 us fixed teardown (full semaphore-file clear). v6:
  * The whole input (128 KB/partition) is SBUF-resident: 9 chunk DMAs
    (7x2-SB + 2x1-SB) all issued up-front on the Sync HWDGE queue, so
    input streams back-to-back from ~7.6 us with no further issue
    dependencies. Final chunks are small to shorten the drain tail.
  * Engine split by measured per-column cost (DVE 2x fp16 tensor_tensor
    0.52 ns/col, ACT 0.83, GPS ~2.1): DVE does re^2 (non-in-place, 2x)
    and the power add (2x) chunk-wide; ACT squares 2560/4096 im cols
    and drains 3/4 of each PSUM tile (it sits closest to PSUM); GPS
    squares the other 1536 im cols and issues the lag-2 chunk stores
    on its SWDGE ring. ~5.5-6 us of engine time per 2-SB chunk, under
    the ~6.5 us/chunk global engine budget, so stores are produced
    fast enough to keep the SDMA engines fed to the end.
  * Drains are issued one chunk late (after the next chunk's forward
    elementwise ops) so their PE waits are free; stores lag two chunks.
  * Each chunk's xh has a C-wide margin holding the previous chunk's
    last block (one 256-col copy per chunk) so every matmul rhs --
    including the cross-superbatch T1 operand -- is one contiguous AP.
- PSUM rules (kept from v5, learned the hard way): a matmul output
  region must not straddle a 2 KB bank boundary, and each half-bank
  holds exactly one accumulation group, opened once and closed once.
"""

import sys
from contextlib import ExitStack

import numpy as np

for _p in ("/opt/trn_rl_repo",):
    if _p not in sys.path:
        sys.path.insert(0, _p)

import concourse.bass as bass  # noqa: E402
import concourse.tile as tile  # noqa: E402
from concourse import bacc, mybir  # noqa: E402
from concourse.bass_utils import run_bass_kernel_spmd  # noqa: E402

T, B, NCORES = 16384, 2048, 8
BL = 128                # time-block size (= PE contraction dim)
NB = T // BL            # 128 time blocks
C = B // NCORES         # 256 channels per core
SBW = 8                 # time blocks per superbatch
NSB = NB // SBW         # 16 superbatches
CHUNKS = (2, 2, 2, 2, 2, 2, 2, 1, 1)   # superbatches per chunk
SBC = SBW * C           # 2048 columns per superbatch (one block-row)
F32 = mybir.dt.float32
F16 = mybir.dt.float16

TRACE = False           # set by test harness for NTFF profiling
LAST_RESULTS = None     # BassKernelResults of the last run (for profiling)

_program_cache = {}


def _reference_operator(bb, aa, n):
    """Exact linear operator of the reference filter on n samples (float64).

    Columns are responses to basis vectors; replicates the reference
    semantics: xf = zero-padded cross-correlation with b, first 5 outputs
    pass through, recurrence y[t] = xf[t] - sum_j a_j y[t-j] from t=5.
    """
    x = np.eye(n)
    xp = np.concatenate([np.zeros((4, n)), x], 0)
    xf = sum(bb[k] * xp[k:k + n] for k in range(5))
    y = xf.copy()
    at = aa[:4]
    for t in range(5, n):
        y[t] = xf[t] - (at[0] * y[t - 4] + at[1] * y[t - 3]
                        + at[2] * y[t - 2] + at[3] * y[t - 1])
    return y


def _build_mats(b32, a32):
    """Returns [BL, 3*BL] fp16: the three lhsT operands packed so the
    weights load with a single contiguous DMA (768 B per partition)."""
    bb = np.asarray(b32, np.float64)
    aa = np.asarray(a32, np.float64)
    M = _reference_operator(bb, aa, 3 * BL)
    L0 = M[0:BL, 0:BL]
    T0 = M[2 * BL:3 * BL, 2 * BL:3 * BL]
    T1 = M[2 * BL:3 * BL, BL:2 * BL]
    # truncation + init-transient leakage must be below fp32 noise
    leak = np.abs(M[2 * BL:3 * BL, 0:BL]).max()
    dev = max(np.abs(M[BL:2 * BL, BL:2 * BL] - T0).max(),
              np.abs(M[BL:2 * BL, 0:BL] - T1).max())
    assert leak < 1e-9 and dev < 1e-9, (leak, dev)

    w = np.empty((BL, 3 * BL), np.float16)
    for j, W in enumerate((L0, T0, T1)):
        w[:, j * BL:(j + 1) * BL] = W.T.astype(np.float16)  # lhsT = W.T
    return np.ascontiguousarray(w)


def _chunk_starts():
    starts, s0 = [], 0
    for L in CHUNKS:
        starts.append(s0)
        s0 += L
    assert s0 == NSB
    return starts


def _build_program():
    nc = bacc.Bacc("TRN2", target_bir_lowering=False, debug=False)
    # input cols per chunk: [re: L*SBC][im: L*SBC], chunk-major
    sig = nc.dram_tensor("sig", [BL, NSB * 2 * SBC], F16,
                         kind="ExternalInput").ap()
    wd = nc.dram_tensor("w", [BL, 3 * BL], F16, kind="ExternalInput").ap()
    yd = nc.dram_tensor("y", [BL, NSB * SBC], F16,
                        kind="ExternalOutput").ap()

    starts = _chunk_starts()
    base = [2 * SBC * s for s in starts]
    NCH = len(CHUNKS)

    with tile.TileContext(nc) as tc, ExitStack() as ctx:
        wpool = ctx.enter_context(tc.tile_pool(name="w", bufs=1))
        wsb = wpool.tile([BL, 3 * BL], F16, tag="w", name="w_sb")
        nc.sync.dma_start(wsb[:], wd)
        w = {"l0": wsb[:, 0:BL], "t0": wsb[:, BL:2 * BL],
             "t1": wsb[:, 2 * BL:3 * BL]}

        sigpool = ctx.enter_context(tc.tile_pool(name="sig", bufs=1))
        p2pool = ctx.enter_context(tc.tile_pool(name="p2", bufs=2))
        xhpool = ctx.enter_context(tc.tile_pool(name="xh", bufs=2))
        yspool = ctx.enter_context(tc.tile_pool(name="ys", bufs=2))
        pspool = ctx.enter_context(tc.tile_pool(name="ps", bufs=2,
                                                space="PSUM"))

        sig_sb = sigpool.tile([BL, NSB * 2 * SBC], F16, tag="sig",
                              name="sig_sb")
        # all input DMAs issued up-front: back-to-back on the Sync ring
        for c, L in enumerate(CHUNKS):
            sp = slice(base[c], base[c] + 2 * L * SBC)
            nc.sync.dma_start(sig_sb[:, sp], sig[:, sp])

        def mm(ps_ap, wt, rhs_ap, start, stop):
            nc.tensor.matmul(ps_ap, w[wt], rhs_ap, start=start, stop=stop,
                             skip_group_check=True)

        MAXW = 2 * SBC                      # widest chunk (L=2) in cols

        def gps_prefetch(c, p2):
            # GPS squares the tail of chunk c's [re|im] block early;
            # its share is the last 5/16 of the chunk's 2*W2 columns
            L = CHUNKS[c]
            W2 = L * SBC
            lo = W2 + 3 * W2 // 8
            nc.gpsimd.tensor_mul(p2[:, lo:2 * W2],
                                 sig_sb[:, base[c] + lo:base[c] + 2 * W2],
                                 sig_sb[:, base[c] + lo:base[c] + 2 * W2])

        pend = []                           # drain entries, FIFO by chunk
        store_q = []                        # [(ys_ap, dram col slice)]
        prev_xh = None
        p2_cur = p2pool.tile([BL, 2 * MAXW], F16, tag="p2", name="p2_0")
        gps_prefetch(0, p2_cur)                    # bootstrap chunk 0
        for c, L in enumerate(CHUNKS):
            W2 = L * SBC
            p2 = p2_cur

            xh = xhpool.tile([BL, MAXW], F16, tag="xh")
            ys = yspool.tile([BL, MAXW], F16, tag="ys")

            # ACT squares the middle band [W2/2 : W2+3*W2/8] (rest of
            # re + front of im) as ONE contiguous instruction
            nc.scalar.activation(p2[:, W2 // 2:W2 + 3 * W2 // 8],
                                 sig_sb[:, base[c] + W2 // 2:
                                        base[c] + W2 + 3 * W2 // 8],
                                 mybir.ActivationFunctionType.Square)
            if c + 1 < NCH:
                p2_next = p2pool.tile([BL, 2 * MAXW], F16, tag="p2",
                                      name="p2_%d" % (c + 1))
                gps_prefetch(c + 1, p2_next)
            else:
                p2_next = None

            # DVE: front half of re^2, then the per-SB power adds (2x)
            nc.vector.tensor_mul(p2[:, 0:W2 // 2],
                                 sig_sb[:, base[c]:base[c] + W2 // 2],
                                 sig_sb[:, base[c]:base[c] + W2 // 2])
            for l in range(L):
                bs = l * SBC
                nc.vector.tensor_add(xh[:, bs:bs + SBC],
                                     p2[:, bs:bs + SBC],
                                     p2[:, W2 + bs:W2 + bs + SBC])

            # matmuls: one [BL, 8C] PSUM tile per superbatch, wide rhs
            for l in range(L):
                s = starts[c] + l
                bs = l * SBC

                def b(i, n=1):
                    return xh[:, bs + i * C:bs + (i + n) * C]

                ps = pspool.tile([BL, 8 * C], F32, tag="ps",
                                 name="ps_%d" % s)
                if s == 0:
                    # exact-init operator L0 for block 0, no cross term
                    mm(ps[:, 0:C], "l0", b(0), True, True)
                    mm(ps[:, C:2 * C], "t0", b(1), True, False)
                    mm(ps[:, C:2 * C], "t1", b(0), False, True)
                elif l == 0:
                    # cross-chunk T1 term: bank 0's group is opened by
                    # the 2C t0, then t1 lands in two pieces (prev
                    # chunk's last block + own block 0) before closing
                    pxh, pl = prev_xh
                    prevC = pxh[:, pl * SBC - C:pl * SBC]
                    mm(ps[:, 0:2 * C], "t0", b(0, 2), True, False)
                    mm(ps[:, 0:C], "t1", prevC, False, False)
                    mm(ps[:, C:2 * C], "t1", b(0), False, True)
                else:
                    mm(ps[:, 0:2 * C], "t0", b(0, 2), True, False)
                    mm(ps[:, 0:2 * C], "t1", b(-1, 2), False, True)
                for r in range(1, 4):       # banks 1..3: blocks 2r..2r+1
                    mm(ps[:, 2 * r * C:(2 * r + 2) * C], "t0",
                       b(2 * r, 2), True, False)
                    mm(ps[:, 2 * r * C:(2 * r + 2) * C], "t1",
                       b(2 * r - 1, 2), False, True)
                pend.append((s, l, ps, ys))

            # drains + store for chunk c-2 (PE finished it last slot);
            # one wide ACT drain instruction per superbatch
            if c >= 2:
                Lp = CHUNKS[c - 2]
                ents, pend = pend[:Lp], pend[Lp:]
                for (ds, dl, dps, dys) in ents:
                    nc.scalar.activation(
                        dys[:, dl * SBC:(dl + 1) * SBC], dps[:],
                        mybir.ActivationFunctionType.Copy)
                ys_ap, cols = store_q.pop(0)
                nc.sync.dma_start(yd[:, cols], ys_ap)

            prev_xh = (xh, L)
            p2_cur = p2_next
            store_q.append((ys[:, 0:W2],
                            slice(starts[c] * SBC, (starts[c] + L) * SBC)))

        # tail: drain + store the last two chunks
        for k, c in enumerate((NCH - 2, NCH - 1)):
            Lp = CHUNKS[c]
            ents, pend = pend[:Lp], pend[Lp:]
            for (ds, dl, dps, dys) in ents:
                nc.scalar.activation(dys[:, dl * SBC:(dl + 1) * SBC],
                                     dps[:],
                                     mybir.ActivationFunctionType.Copy)
            ys_ap, cols = store_q.pop(0)
            if k == 0:
                nc.sync.dma_start(yd[:, cols], ys_ap)
            else:
                # final store split across both DMA paths for latency
                n = cols.stop - cols.start
                nc.sync.dma_start(yd[:, cols.start:cols.start + n // 2],
                                  ys_ap[:, 0:n // 2])
                nc.gpsimd.dma_start(yd[:, cols.start + n // 2:cols.stop],
                                    ys_ap[:, n // 2:n])
        assert not pend and not store_q

    nc.compile()
    return nc


def kernel(signal, b, a):
    global LAST_RESULTS
    signal = np.asarray(signal)
    assert signal.shape == (2, T, B), signal.shape

    wmat = _build_mats(np.asarray(b), np.asarray(a))

    if "prog" not in _program_cache:
        _program_cache["prog"] = _build_program()
    nc = _program_cache["prog"]

    starts = _chunk_starts()
    # pack to per-core chunk-major fp16 layout:
    # [core, p, chunk{ re[l,b,ch] | im[l,b,ch] }]
    x = signal.reshape(2, NSB, SBW, BL, NCORES, C)
    parts = []
    for c, L in enumerate(CHUNKS):
        xs = x[:, starts[c]:starts[c] + L]        # [2, L, SBW, BL, 8, C]
        parts.append(xs.transpose(4, 3, 0, 1, 2, 5).reshape(
            NCORES, BL, 2 * L * SBW * C))
    pk = np.ascontiguousarray(np.concatenate(parts, axis=2),
                              dtype=np.float16)   # [8, BL, NSB*2*SBC]

    in_maps = [{"sig": pk[c], "w": wmat} for c in range(NCORES)]

    res = run_bass_kernel_spmd(nc, in_maps, core_ids=list(range(NCORES)),
                               trace=TRACE)
    LAST_RESULTS = res

    out = np.empty((T, B), np.float32)
    for c in range(NCORES):
        yc = np.asarray(res.results[c]["y"])      # [BL, NSB*SBC]
        yc = yc.reshape(BL, NSB, SBW, C).transpose(1, 2, 0, 3)
        out[:, c * C:(c + 1) * C] = yc.reshape(T, C).astype(np.float32)
    return out


# revision 18
# speedup vs baseline: 1.1318x; 1.1318x over previous
"""Trainium2 Bass kernel for nn_LowpassDetector.

Computes: power = re^2 + im^2, 5-tap FIR (b), order-4 IIR recurrence (a)
along time, for signal [2, T=16384, B=2048] -> y [T, B].

Strategy: the FIR+IIR cascade is LTI with all poles at radius <= 0.758,
so the combined impulse response decays below 1e-15 within 128 taps.
The whole filter is therefore exactly (to fp32) a block-Toeplitz matmul:
  y_blk[b] = T0 @ x_blk[b] + T1 @ x_blk[b-1]     (b >= 1)
  y_blk[0] = L0 @ x_blk[0]
with L0 the exact 128x128 operator of the reference recurrence
(including its "first 5 samples pass through" initial condition), built
on the host in float64. Channels (2048) are sharded 256 per core across
8 cores; time blocks of 128 map to the TensorEngine contraction dim.

v6 design (from v5's ~91-98 us; trace-driven):
- All I/O fp16 (fp8-e3m4/e4m3 input was simulated on host and fails the
  2e-2 max-rel budget at 3-6.6e-2: the metric is tail-dominated and
  fp8's coarse ulp at large |x| survives the filter). 25.3 MB/core at
  the measured ~420 GB/s sustained = ~60 us hard DMA floor.
- v5's trace: ~7 us fixed preamble, 16 serial input-DMA issues at
  ~0.6 us each, a 23 us tail at 1/3 rate where stores trailed per-SB
  compute, and ~8 us fixed teardown (full semaphore-file clear). v6:
  * The whole input (128 KB/partition) is SBUF-resident: 9 chunk DMAs
    (7x2-SB + 2x1-SB) all issued up-front on the Sync HWDGE queue, so
    input streams back-to-back from ~7.6 us with no further issue
    dependencies. Final chunks are small to shorten the drain tail.
  * Engine split by measured per-column cost (DVE 2x fp16 tensor_tensor
    0.52 ns/col, ACT 0.83, GPS ~2.1): DVE does re^2 (non-in-place, 2x)
    and the power add (2x) chunk-wide; ACT squares 2560/4096 im cols
    and drains 3/4 of each PSUM tile (it sits closest to PSUM); GPS
    squares the other 1536 im cols and issues the lag-2 chunk stores
    on its SWDGE ring. ~5.5-6 us of engine time per 2-SB chunk, under
    the ~6.5 us/chunk global engine budget, so stores are produced
    fast enough to keep the SDMA engines fed to the end.
  * Drains are issued one chunk late (after the next chunk's forward
    elementwise ops) so their PE waits are free; stores lag two chunks.
  * Each chunk's xh has a C-wide margin holding the previous chunk's
    last block (one 256-col copy per chunk) so every matmul rhs --
    including the cross-superbatch T1 operand -- is one contiguous AP.
- PSUM rules (kept from v5, learned the hard way): a matmul output
  region must not straddle a 2 KB bank boundary, and each half-bank
  holds exactly one accumulation group, opened once and closed once.
"""

import sys
from contextlib import ExitStack

import numpy as np

for _p in ("/opt/trn_rl_repo",):
    if _p not in sys.path:
        sys.path.insert(0, _p)

import concourse.bass as bass  # noqa: E402
import concourse.tile as tile  # noqa: E402
from concourse import bacc, mybir  # noqa: E402
from concourse.bass_utils import run_bass_kernel_spmd  # noqa: E402

T, B, NCORES = 16384, 2048, 8
BL = 128                # time-block size (= PE contraction dim)
NB = T // BL            # 128 time blocks
C = B // NCORES         # 256 channels per core
SBW = 8                 # time blocks per superbatch
NSB = NB // SBW         # 16 superbatches
CHUNKS = (2, 2, 2, 2, 2, 2, 2, 1, 1)   # superbatches per chunk
SBC = SBW * C           # 2048 columns per superbatch (one block-row)
F32 = mybir.dt.float32
F16 = mybir.dt.float16

TRACE = False           # set by test harness for NTFF profiling
LAST_RESULTS = None     # BassKernelResults of the last run (for profiling)

_program_cache = {}


def _reference_operator(bb, aa, n):
    """Exact linear operator of the reference filter on n samples (float64).

    Columns are responses to basis vectors; replicates the reference
    semantics: xf = zero-padded cross-correlation with b, first 5 outputs
    pass through, recurrence y[t] = xf[t] - sum_j a_j y[t-j] from t=5.
    """
    x = np.eye(n)
    xp = np.concatenate([np.zeros((4, n)), x], 0)
    xf = sum(bb[k] * xp[k:k + n] for k in range(5))
    y = xf.copy()
    at = aa[:4]
    for t in range(5, n):
        y[t] = xf[t] - (at[0] * y[t - 4] + at[1] * y[t - 3]
                        + at[2] * y[t - 2] + at[3] * y[t - 1])
    return y


def _build_mats(b32, a32):
    """Returns [BL, 3*BL] fp16: the three lhsT operands packed so the
    weights load with a single contiguous DMA (768 B per partition)."""
    bb = np.asarray(b32, np.float64)
    aa = np.asarray(a32, np.float64)
    M = _reference_operator(bb, aa, 3 * BL)
    L0 = M[0:BL, 0:BL]
    T0 = M[2 * BL:3 * BL, 2 * BL:3 * BL]
    T1 = M[2 * BL:3 * BL, BL:2 * BL]
    # truncation + init-transient leakage must be below fp32 noise
    leak = np.abs(M[2 * BL:3 * BL, 0:BL]).max()
    dev = max(np.abs(M[BL:2 * BL, BL:2 * BL] - T0).max(),
              np.abs(M[BL:2 * BL, 0:BL] - T1).max())
    assert leak < 1e-9 and dev < 1e-9, (leak, dev)

    w = np.empty((BL, 3 * BL), np.float16)
    for j, W in enumerate((L0, T0, T1)):
        w[:, j * BL:(j + 1) * BL] = W.T.astype(np.float16)  # lhsT = W.T
    return np.ascontiguousarray(w)


def _chunk_starts():
    starts, s0 = [], 0
    for L in CHUNKS:
        starts.append(s0)
        s0 += L
    assert s0 == NSB
    return starts


def _build_program():
    nc = bacc.Bacc("TRN2", target_bir_lowering=False, debug=False)
    # input cols per chunk: [re: L*SBC][im: L*SBC], chunk-major
    sig = nc.dram_tensor("sig", [BL, NSB * 2 * SBC], F16,
                         kind="ExternalInput").ap()
    wd = nc.dram_tensor("w", [BL, 3 * BL], F16, kind="ExternalInput").ap()
    yd = nc.dram_tensor("y", [BL, NSB * SBC], F16,
                        kind="ExternalOutput").ap()

    starts = _chunk_starts()
    base = [2 * SBC * s for s in starts]
    NCH = len(CHUNKS)

    with tile.TileContext(nc) as tc, ExitStack() as ctx:
        wpool = ctx.enter_context(tc.tile_pool(name="w", bufs=1))
        wsb = wpool.tile([BL, 3 * BL], F16, tag="w", name="w_sb")
        nc.sync.dma_start(wsb[:], wd)
        w = {"l0": wsb[:, 0:BL], "t0": wsb[:, BL:2 * BL],
             "t1": wsb[:, 2 * BL:3 * BL]}

        sigpool = ctx.enter_context(tc.tile_pool(name="sig", bufs=1))
        p2pool = ctx.enter_context(tc.tile_pool(name="p2", bufs=2))
        xhpool = ctx.enter_context(tc.tile_pool(name="xh", bufs=2))
        yspool = ctx.enter_context(tc.tile_pool(name="ys", bufs=2))
        pspool = ctx.enter_context(tc.tile_pool(name="ps", bufs=2,
                                                space="PSUM"))

        sig_sb = sigpool.tile([BL, NSB * 2 * SBC], F16, tag="sig",
                              name="sig_sb")
        # all input DMAs issued up-front: back-to-back on the Sync ring
        for c, L in enumerate(CHUNKS):
            sp = slice(base[c], base[c] + 2 * L * SBC)
            nc.sync.dma_start(sig_sb[:, sp], sig[:, sp])

        def mm(ps_ap, wt, rhs_ap, start, stop):
            nc.tensor.matmul(ps_ap, w[wt], rhs_ap, start=start, stop=stop,
                             skip_group_check=True)

        MAXW = 2 * SBC                      # widest chunk (L=2) in cols

        pend = []                           # drain entries, FIFO by chunk
        store_q = []                        # [(ys_ap, dram col slice)]
        prev_xh = None
        for c, L in enumerate(CHUNKS):
            W2 = L * SBC
            p2 = p2pool.tile([BL, 2 * MAXW], F16, tag="p2",
                             name="p2_%d" % c)

            xh = xhpool.tile([BL, MAXW], F16, tag="xh")
            ys = yspool.tile([BL, MAXW], F16, tag="ys")

            # GPS is kept OFF the critical chain (stores only): squares
            # split DVE 3/4 (2x tensor_tensor, 0.59 ns/col measured)
            # and ACT 1/4 (the last superbatch's im half, so each add
            # depends only on work finishing early in its slot)
            nc.scalar.activation(p2[:, 3 * W2 // 2:2 * W2],
                                 sig_sb[:, base[c] + 3 * W2 // 2:
                                        base[c] + 2 * W2],
                                 mybir.ActivationFunctionType.Square)
            nc.vector.tensor_mul(p2[:, 0:3 * W2 // 2],
                                 sig_sb[:, base[c]:base[c] + 3 * W2 // 2],
                                 sig_sb[:, base[c]:base[c] + 3 * W2 // 2])
            for l in range(L):
                bs = l * SBC
                nc.vector.tensor_add(xh[:, bs:bs + SBC],
                                     p2[:, bs:bs + SBC],
                                     p2[:, W2 + bs:W2 + bs + SBC])

            # matmuls: one [BL, 8C] PSUM tile per superbatch, wide rhs
            for l in range(L):
                s = starts[c] + l
                bs = l * SBC

                def b(i, n=1):
                    return xh[:, bs + i * C:bs + (i + n) * C]

                ps = pspool.tile([BL, 8 * C], F32, tag="ps",
                                 name="ps_%d" % s)
                if s == 0:
                    # exact-init operator L0 for block 0, no cross term
                    mm(ps[:, 0:C], "l0", b(0), True, True)
                    mm(ps[:, C:2 * C], "t0", b(1), True, False)
                    mm(ps[:, C:2 * C], "t1", b(0), False, True)
                elif l == 0:
                    # cross-chunk T1 term: bank 0's group is opened by
                    # the 2C t0, then t1 lands in two pieces (prev
                    # chunk's last block + own block 0) before closing
                    pxh, pl = prev_xh
                    prevC = pxh[:, pl * SBC - C:pl * SBC]
                    mm(ps[:, 0:2 * C], "t0", b(0, 2), True, False)
                    mm(ps[:, 0:C], "t1", prevC, False, False)
                    mm(ps[:, C:2 * C], "t1", b(0), False, True)
                else:
                    mm(ps[:, 0:2 * C], "t0", b(0, 2), True, False)
                    mm(ps[:, 0:2 * C], "t1", b(-1, 2), False, True)
                for r in range(1, 4):       # banks 1..3: blocks 2r..2r+1
                    mm(ps[:, 2 * r * C:(2 * r + 2) * C], "t0",
                       b(2 * r, 2), True, False)
                    mm(ps[:, 2 * r * C:(2 * r + 2) * C], "t1",
                       b(2 * r - 1, 2), False, True)
                pend.append((s, l, ps, ys))

            # drains + store for chunk c-2 (PE finished it last slot);
            # one wide ACT drain instruction per superbatch
            if c >= 2:
                Lp = CHUNKS[c - 2]
                ents, pend = pend[:Lp], pend[Lp:]
                for (ds, dl, dps, dys) in ents:
                    nc.scalar.activation(
                        dys[:, dl * SBC:(dl + 1) * SBC], dps[:],
                        mybir.ActivationFunctionType.Copy)
                ys_ap, cols = store_q.pop(0)
                nc.gpsimd.dma_start(yd[:, cols], ys_ap)

            prev_xh = (xh, L)
            store_q.append((ys[:, 0:W2],
                            slice(starts[c] * SBC, (starts[c] + L) * SBC)))

        # tail: drain + store the last two chunks
        for k, c in enumerate((NCH - 2, NCH - 1)):
            Lp = CHUNKS[c]
            ents, pend = pend[:Lp], pend[Lp:]
            for (ds, dl, dps, dys) in ents:
                nc.scalar.activation(dys[:, dl * SBC:(dl + 1) * SBC],
                                     dps[:],
                                     mybir.ActivationFunctionType.Copy)
            ys_ap, cols = store_q.pop(0)
            if k == 0:
                nc.gpsimd.dma_start(yd[:, cols], ys_ap)
            else:
                # final store split across both DMA paths for latency
                n = cols.stop - cols.start
                nc.sync.dma_start(yd[:, cols.start:cols.start + n // 2],
                                  ys_ap[:, 0:n // 2])
                nc.gpsimd.dma_start(yd[:, cols.start + n // 2:cols.stop],
                                    ys_ap[:, n // 2:n])
        assert not pend and not store_q

    nc.compile()
    return nc


def kernel(signal, b, a):
    global LAST_RESULTS
    signal = np.asarray(signal)
    assert signal.shape == (2, T, B), signal.shape

    wmat = _build_mats(np.asarray(b), np.asarray(a))

    if "prog" not in _program_cache:
        _program_cache["prog"] = _build_program()
    nc = _program_cache["prog"]

    starts = _chunk_starts()
    # pack to per-core chunk-major fp16 layout:
    # [core, p, chunk{ re[l,b,ch] | im[l,b,ch] }]
    x = signal.reshape(2, NSB, SBW, BL, NCORES, C)
    parts = []
    for c, L in enumerate(CHUNKS):
        xs = x[:, starts[c]:starts[c] + L]        # [2, L, SBW, BL, 8, C]
        parts.append(xs.transpose(4, 3, 0, 1, 2, 5).reshape(
            NCORES, BL, 2 * L * SBW * C))
    pk = np.ascontiguousarray(np.concatenate(parts, axis=2),
                              dtype=np.float16)   # [8, BL, NSB*2*SBC]

    in_maps = [{"sig": pk[c], "w": wmat} for c in range(NCORES)]

    res = run_bass_kernel_spmd(nc, in_maps, core_ids=list(range(NCORES)),
                               trace=TRACE)
    LAST_RESULTS = res

    out = np.empty((T, B), np.float32)
    for c in range(NCORES):
        yc = np.asarray(res.results[c]["y"])      # [BL, NSB*SBC]
        yc = yc.reshape(BL, NSB, SBW, C).transpose(1, 2, 0, 3)
        out[:, c * C:(c + 1) * C] = yc.reshape(T, C).astype(np.float32)
    return out


# revision 19
# speedup vs baseline: 1.1360x; 1.0037x over previous
"""Trainium2 Bass kernel for nn_LowpassDetector.

Computes: power = re^2 + im^2, 5-tap FIR (b), order-4 IIR recurrence (a)
along time, for signal [2, T=16384, B=2048] -> y [T, B].

Strategy: the FIR+IIR cascade is LTI with all poles at radius <= 0.758,
so the combined impulse response decays below 1e-15 within 128 taps.
The whole filter is therefore exactly (to fp32) a block-Toeplitz matmul:
  y_blk[b] = T0 @ x_blk[b] + T1 @ x_blk[b-1]     (b >= 1)
  y_blk[0] = L0 @ x_blk[0]
with L0 the exact 128x128 operator of the reference recurrence
(including its "first 5 samples pass through" initial condition), built
on the host in float64. Channels (2048) are sharded 256 per core across
8 cores; time blocks of 128 map to the TensorEngine contraction dim.

v6 design (from v5's ~91-98 us; trace-driven):
- All I/O fp16 (fp8-e3m4/e4m3 input was simulated on host and fails the
  2e-2 max-rel budget at 3-6.6e-2: the metric is tail-dominated and
  fp8's coarse ulp at large |x| survives the filter). 25.3 MB/core at
  the measured ~420 GB/s sustained = ~60 us hard DMA floor.
- v5's trace: ~7 us fixed preamble, 16 serial input-DMA issues at
  ~0.6 us each, a 23 us tail at 1/3 rate where stores trailed per-SB
  compute, and ~8 us fixed teardown (full semaphore-file clear). v6:
  * The whole input (128 KB/partition) is SBUF-resident: 9 chunk DMAs
    (7x2-SB + 2x1-SB) all issued up-front on the Sync HWDGE queue, so
    input streams back-to-back from ~7.6 us with no further issue
    dependencies. Final chunks are small to shorten the drain tail.
  * Engine split by measured per-column cost (DVE 2x fp16 tensor_tensor
    0.52 ns/col, ACT 0.83, GPS ~2.1): DVE does re^2 (non-in-place, 2x)
    and the power add (2x) chunk-wide; ACT squares 2560/4096 im cols
    and drains 3/4 of each PSUM tile (it sits closest to PSUM); GPS
    squares the other 1536 im cols and issues the lag-2 chunk stores
    on its SWDGE ring. ~5.5-6 us of engine time per 2-SB chunk, under
    the ~6.5 us/chunk global engine budget, so stores are produced
    fast enough to keep the SDMA engines fed to the end.
  * Drains are issued one chunk late (after the next chunk's forward
    elementwise ops) so their PE waits are free; stores lag two chunks.
  * Each chunk's xh has a C-wide margin holding the previous chunk's
    last block (one 256-col copy per chunk) so every matmul rhs --
    including the cross-superbatch T1 operand -- is one contiguous AP.
- PSUM rules (kept from v5, learned the hard way): a matmul output
  region must not straddle a 2 KB bank boundary, and each half-bank
  holds exactly one accumulation group, opened once and closed once.
"""

import sys
from contextlib import ExitStack

import numpy as np

for _p in ("/opt/trn_rl_repo",):
    if _p not in sys.path:
        sys.path.insert(0, _p)

import concourse.bass as bass  # noqa: E402
import concourse.tile as tile  # noqa: E402
from concourse import bacc, mybir  # noqa: E402
from concourse.bass_utils import run_bass_kernel_spmd  # noqa: E402

T, B, NCORES = 16384, 2048, 8
BL = 128                # time-block size (= PE contraction dim)
NB = T // BL            # 128 time blocks
C = B // NCORES         # 256 channels per core
SBW = 8                 # time blocks per superbatch
NSB = NB // SBW         # 16 superbatches
CHUNKS = (2, 2, 2, 2, 2, 2, 2, 1, 1)   # superbatches per chunk
SBC = SBW * C           # 2048 columns per superbatch (one block-row)
F32 = mybir.dt.float32
F16 = mybir.dt.float16

TRACE = False           # set by test harness for NTFF profiling
LAST_RESULTS = None     # BassKernelResults of the last run (for profiling)

_program_cache = {}


def _reference_operator(bb, aa, n):
    """Exact linear operator of the reference filter on n samples (float64).

    Columns are responses to basis vectors; replicates the reference
    semantics: xf = zero-padded cross-correlation with b, first 5 outputs
    pass through, recurrence y[t] = xf[t] - sum_j a_j y[t-j] from t=5.
    """
    x = np.eye(n)
    xp = np.concatenate([np.zeros((4, n)), x], 0)
    xf = sum(bb[k] * xp[k:k + n] for k in range(5))
    y = xf.copy()
    at = aa[:4]
    for t in range(5, n):
        y[t] = xf[t] - (at[0] * y[t - 4] + at[1] * y[t - 3]
                        + at[2] * y[t - 2] + at[3] * y[t - 1])
    return y


def _build_mats(b32, a32):
    """Returns [BL, 3*BL] fp16: the three lhsT operands packed so the
    weights load with a single contiguous DMA (768 B per partition)."""
    bb = np.asarray(b32, np.float64)
    aa = np.asarray(a32, np.float64)
    M = _reference_operator(bb, aa, 3 * BL)
    L0 = M[0:BL, 0:BL]
    T0 = M[2 * BL:3 * BL, 2 * BL:3 * BL]
    T1 = M[2 * BL:3 * BL, BL:2 * BL]
    # truncation + init-transient leakage must be below fp32 noise
    leak = np.abs(M[2 * BL:3 * BL, 0:BL]).max()
    dev = max(np.abs(M[BL:2 * BL, BL:2 * BL] - T0).max(),
              np.abs(M[BL:2 * BL, 0:BL] - T1).max())
    assert leak < 1e-9 and dev < 1e-9, (leak, dev)

    w = np.empty((BL, 3 * BL), np.float16)
    for j, W in enumerate((L0, T0, T1)):
        w[:, j * BL:(j + 1) * BL] = W.T.astype(np.float16)  # lhsT = W.T
    return np.ascontiguousarray(w)


def _chunk_starts():
    starts, s0 = [], 0
    for L in CHUNKS:
        starts.append(s0)
        s0 += L
    assert s0 == NSB
    return starts


def _build_program():
    nc = bacc.Bacc("TRN2", target_bir_lowering=False, debug=False)
    # input cols per chunk: [re: L*SBC][im: L*SBC], chunk-major
    sig = nc.dram_tensor("sig", [BL, NSB * 2 * SBC], F16,
                         kind="ExternalInput").ap()
    wd = nc.dram_tensor("w", [BL, 3 * BL], F16, kind="ExternalInput").ap()
    yd = nc.dram_tensor("y", [BL, NSB * SBC], F16,
                        kind="ExternalOutput").ap()

    starts = _chunk_starts()
    base = [2 * SBC * s for s in starts]
    NCH = len(CHUNKS)

    with tile.TileContext(nc) as tc, ExitStack() as ctx:
        wpool = ctx.enter_context(tc.tile_pool(name="w", bufs=1))
        wsb = wpool.tile([BL, 3 * BL], F16, tag="w", name="w_sb")
        nc.sync.dma_start(wsb[:], wd)
        w = {"l0": wsb[:, 0:BL], "t0": wsb[:, BL:2 * BL],
             "t1": wsb[:, 2 * BL:3 * BL]}

        sigpool = ctx.enter_context(tc.tile_pool(name="sig", bufs=1))
        p2pool = ctx.enter_context(tc.tile_pool(name="p2", bufs=2))
        xhpool = ctx.enter_context(tc.tile_pool(name="xh", bufs=3))
        yspool = ctx.enter_context(tc.tile_pool(name="ys", bufs=2))
        pspool = ctx.enter_context(tc.tile_pool(name="ps", bufs=2,
                                                space="PSUM"))

        sig_sb = sigpool.tile([BL, NSB * 2 * SBC], F16, tag="sig",
                              name="sig_sb")
        # all input DMAs issued up-front: back-to-back on the Sync ring
        for c, L in enumerate(CHUNKS):
            sp = slice(base[c], base[c] + 2 * L * SBC)
            nc.sync.dma_start(sig_sb[:, sp], sig[:, sp])

        def mm(ps_ap, wt, rhs_ap, start, stop):
            nc.tensor.matmul(ps_ap, w[wt], rhs_ap, start=start, stop=stop,
                             skip_group_check=True)

        MAXW = 2 * SBC                      # widest chunk (L=2) in cols

        pend = []                           # drain entries, FIFO by chunk
        store_q = []                        # [(ys_ap, dram col slice)]
        prev_xh = None
        for c, L in enumerate(CHUNKS):
            W2 = L * SBC
            p2 = p2pool.tile([BL, 2 * MAXW], F16, tag="p2",
                             name="p2_%d" % c)

            xh = xhpool.tile([BL, MAXW], F16, tag="xh")
            ys = yspool.tile([BL, MAXW], F16, tag="ys")

            # GPS is kept OFF the critical chain (stores only): squares
            # split DVE 3/4 (2x tensor_tensor, 0.59 ns/col measured)
            # and ACT 1/4 (the last superbatch's im half, so each add
            # depends only on work finishing early in its slot)
            nc.scalar.activation(p2[:, 3 * W2 // 2:2 * W2],
                                 sig_sb[:, base[c] + 3 * W2 // 2:
                                        base[c] + 2 * W2],
                                 mybir.ActivationFunctionType.Square)
            nc.vector.tensor_mul(p2[:, 0:3 * W2 // 2],
                                 sig_sb[:, base[c]:base[c] + 3 * W2 // 2],
                                 sig_sb[:, base[c]:base[c] + 3 * W2 // 2])
            for l in range(L):
                bs = l * SBC
                nc.vector.tensor_add(xh[:, bs:bs + SBC],
                                     p2[:, bs:bs + SBC],
                                     p2[:, W2 + bs:W2 + bs + SBC])

            # matmuls: one [BL, 8C] PSUM tile per superbatch, wide rhs
            for l in range(L):
                s = starts[c] + l
                bs = l * SBC

                def b(i, n=1):
                    return xh[:, bs + i * C:bs + (i + n) * C]

                ps = pspool.tile([BL, 8 * C], F32, tag="ps",
                                 name="ps_%d" % s)
                if s == 0:
                    # exact-init operator L0 for block 0, no cross term
                    mm(ps[:, 0:C], "l0", b(0), True, True)
                    mm(ps[:, C:2 * C], "t0", b(1), True, False)
                    mm(ps[:, C:2 * C], "t1", b(0), False, True)
                elif l == 0:
                    # cross-chunk T1 term: bank 0's group is opened by
                    # the 2C t0, then t1 lands in two pieces (prev
                    # chunk's last block + own block 0) before closing
                    pxh, pl = prev_xh
                    prevC = pxh[:, pl * SBC - C:pl * SBC]
                    mm(ps[:, 0:2 * C], "t0", b(0, 2), True, False)
                    mm(ps[:, 0:C], "t1", prevC, False, False)
                    mm(ps[:, C:2 * C], "t1", b(0), False, True)
                else:
                    mm(ps[:, 0:2 * C], "t0", b(0, 2), True, False)
                    mm(ps[:, 0:2 * C], "t1", b(-1, 2), False, True)
                # banks 1..3: all t0 opens first, then all t1 closes --
                # one weight switch instead of one per matmul
                for r in range(1, 4):
                    mm(ps[:, 2 * r * C:(2 * r + 2) * C], "t0",
                       b(2 * r, 2), True, False)
                for r in range(1, 4):
                    mm(ps[:, 2 * r * C:(2 * r + 2) * C], "t1",
                       b(2 * r - 1, 2), False, True)
                pend.append((s, l, ps, ys))

            # drains + store for chunk c-2 (PE finished it last slot);
            # one wide ACT drain instruction per superbatch
            if c >= 2:
                Lp = CHUNKS[c - 2]
                ents, pend = pend[:Lp], pend[Lp:]
                for (ds, dl, dps, dys) in ents:
                    nc.scalar.activation(
                        dys[:, dl * SBC:(dl + 1) * SBC], dps[:],
                        mybir.ActivationFunctionType.Copy)
                ys_ap, cols = store_q.pop(0)
                nc.gpsimd.dma_start(yd[:, cols], ys_ap)

            prev_xh = (xh, L)
            store_q.append((ys[:, 0:W2],
                            slice(starts[c] * SBC, (starts[c] + L) * SBC)))

        # tail: drain + store the last two chunks
        for k, c in enumerate((NCH - 2, NCH - 1)):
            Lp = CHUNKS[c]
            ents, pend = pend[:Lp], pend[Lp:]
            for (ds, dl, dps, dys) in ents:
                nc.scalar.activation(dys[:, dl * SBC:(dl + 1) * SBC],
                                     dps[:],
                                     mybir.ActivationFunctionType.Copy)
            ys_ap, cols = store_q.pop(0)
            if k == 0:
                nc.gpsimd.dma_start(yd[:, cols], ys_ap)
            else:
                # final store split across both DMA paths for latency
                n = cols.stop - cols.start
                nc.sync.dma_start(yd[:, cols.start:cols.start + n // 2],
                                  ys_ap[:, 0:n // 2])
                nc.gpsimd.dma_start(yd[:, cols.start + n // 2:cols.stop],
                                    ys_ap[:, n // 2:n])
        assert not pend and not store_q

    nc.compile()
    return nc


def kernel(signal, b, a):
    global LAST_RESULTS
    signal = np.asarray(signal)
    assert signal.shape == (2, T, B), signal.shape

    wmat = _build_mats(np.asarray(b), np.asarray(a))

    if "prog" not in _program_cache:
        _program_cache["prog"] = _build_program()
    nc = _program_cache["prog"]

    starts = _chunk_starts()
    # pack to per-core chunk-major fp16 layout:
    # [core, p, chunk{ re[l,b,ch] | im[l,b,ch] }]
    x = signal.reshape(2, NSB, SBW, BL, NCORES, C)
    parts = []
    for c, L in enumerate(CHUNKS):
        xs = x[:, starts[c]:starts[c] + L]        # [2, L, SBW, BL, 8, C]
        parts.append(xs.transpose(4, 3, 0, 1, 2, 5).reshape(
            NCORES, BL, 2 * L * SBW * C))
    pk = np.ascontiguousarray(np.concatenate(parts, axis=2),
                              dtype=np.float16)   # [8, BL, NSB*2*SBC]

    in_maps = [{"sig": pk[c], "w": wmat} for c in range(NCORES)]

    res = run_bass_kernel_spmd(nc, in_maps, core_ids=list(range(NCORES)),
                               trace=TRACE)
    LAST_RESULTS = res

    out = np.empty((T, B), np.float32)
    for c in range(NCORES):
        yc = np.asarray(res.results[c]["y"])      # [BL, NSB*SBC]
        yc = yc.reshape(BL, NSB, SBW, C).transpose(1, 2, 0, 3)
        out[:, c * C:(c + 1) * C] = yc.reshape(T, C).astype(np.float32)
    return out


# revision 20
# speedup vs baseline: 1.1553x; 1.0170x over previous
"""Trainium2 Bass kernel for nn_LowpassDetector.

Computes: power = re^2 + im^2, 5-tap FIR (b), order-4 IIR recurrence (a)
along time, for signal [2, T=16384, B=2048] -> y [T, B].

Strategy: the FIR+IIR cascade is LTI with all poles at radius <= 0.758,
so the combined impulse response decays below 1e-15 within 128 taps.
The whole filter is therefore exactly (to fp32) a block-Toeplitz matmul:
  y_blk[b] = T0 @ x_blk[b] + T1 @ x_blk[b-1]     (b >= 1)
  y_blk[0] = L0 @ x_blk[0]
with L0 the exact 128x128 operator of the reference recurrence
(including its "first 5 samples pass through" initial condition), built
on the host in float64. Channels (2048) are sharded 256 per core across
8 cores; time blocks of 128 map to the TensorEngine contraction dim.

v6 design (from v5's ~91-98 us; trace-driven):
- All I/O fp16 (fp8-e3m4/e4m3 input was simulated on host and fails the
  2e-2 max-rel budget at 3-6.6e-2: the metric is tail-dominated and
  fp8's coarse ulp at large |x| survives the filter). 25.3 MB/core at
  the measured ~420 GB/s sustained = ~60 us hard DMA floor.
- v5's trace: ~7 us fixed preamble, 16 serial input-DMA issues at
  ~0.6 us each, a 23 us tail at 1/3 rate where stores trailed per-SB
  compute, and ~8 us fixed teardown (full semaphore-file clear). v6:
  * The whole input (128 KB/partition) is SBUF-resident: 9 chunk DMAs
    (7x2-SB + 2x1-SB) all issued up-front on the Sync HWDGE queue, so
    input streams back-to-back from ~7.6 us with no further issue
    dependencies. Final chunks are small to shorten the drain tail.
  * Engine split by measured per-column cost (DVE 2x fp16 tensor_tensor
    0.52 ns/col, ACT 0.83, GPS ~2.1): DVE does re^2 (non-in-place, 2x)
    and the power add (2x) chunk-wide; ACT squares 2560/4096 im cols
    and drains 3/4 of each PSUM tile (it sits closest to PSUM); GPS
    squares the other 1536 im cols and issues the lag-2 chunk stores
    on its SWDGE ring. ~5.5-6 us of engine time per 2-SB chunk, under
    the ~6.5 us/chunk global engine budget, so stores are produced
    fast enough to keep the SDMA engines fed to the end.
  * Drains are issued one chunk late (after the next chunk's forward
    elementwise ops) so their PE waits are free; stores lag two chunks.
  * Each chunk's xh has a C-wide margin holding the previous chunk's
    last block (one 256-col copy per chunk) so every matmul rhs --
    including the cross-superbatch T1 operand -- is one contiguous AP.
- PSUM rules (kept from v5, learned the hard way): a matmul output
  region must not straddle a 2 KB bank boundary, and each half-bank
  holds exactly one accumulation group, opened once and closed once.
"""

import sys
from contextlib import ExitStack

import numpy as np

for _p in ("/opt/trn_rl_repo",):
    if _p not in sys.path:
        sys.path.insert(0, _p)

import concourse.bass as bass  # noqa: E402
import concourse.tile as tile  # noqa: E402
from concourse import bacc, mybir  # noqa: E402
from concourse.bass_utils import run_bass_kernel_spmd  # noqa: E402

T, B, NCORES = 16384, 2048, 8
BL = 128                # time-block size (= PE contraction dim)
NB = T // BL            # 128 time blocks
C = B // NCORES         # 256 channels per core
SBW = 8                 # time blocks per superbatch
NSB = NB // SBW         # 16 superbatches
CHUNKS = (1, 2, 2, 2, 2, 2, 2, 1, 1, 1)   # superbatches per chunk
SBC = SBW * C           # 2048 columns per superbatch (one block-row)
F32 = mybir.dt.float32
F16 = mybir.dt.float16

TRACE = False           # set by test harness for NTFF profiling
LAST_RESULTS = None     # BassKernelResults of the last run (for profiling)

_program_cache = {}


def _reference_operator(bb, aa, n):
    """Exact linear operator of the reference filter on n samples (float64).

    Columns are responses to basis vectors; replicates the reference
    semantics: xf = zero-padded cross-correlation with b, first 5 outputs
    pass through, recurrence y[t] = xf[t] - sum_j a_j y[t-j] from t=5.
    """
    x = np.eye(n)
    xp = np.concatenate([np.zeros((4, n)), x], 0)
    xf = sum(bb[k] * xp[k:k + n] for k in range(5))
    y = xf.copy()
    at = aa[:4]
    for t in range(5, n):
        y[t] = xf[t] - (at[0] * y[t - 4] + at[1] * y[t - 3]
                        + at[2] * y[t - 2] + at[3] * y[t - 1])
    return y


def _build_mats(b32, a32):
    """Returns [BL, 3*BL] fp16: the three lhsT operands packed so the
    weights load with a single contiguous DMA (768 B per partition)."""
    bb = np.asarray(b32, np.float64)
    aa = np.asarray(a32, np.float64)
    M = _reference_operator(bb, aa, 3 * BL)
    L0 = M[0:BL, 0:BL]
    T0 = M[2 * BL:3 * BL, 2 * BL:3 * BL]
    T1 = M[2 * BL:3 * BL, BL:2 * BL]
    # truncation + init-transient leakage must be below fp32 noise
    leak = np.abs(M[2 * BL:3 * BL, 0:BL]).max()
    dev = max(np.abs(M[BL:2 * BL, BL:2 * BL] - T0).max(),
              np.abs(M[BL:2 * BL, 0:BL] - T1).max())
    assert leak < 1e-9 and dev < 1e-9, (leak, dev)

    w = np.empty((BL, 3 * BL), np.float16)
    for j, W in enumerate((L0, T0, T1)):
        w[:, j * BL:(j + 1) * BL] = W.T.astype(np.float16)  # lhsT = W.T
    return np.ascontiguousarray(w)


def _chunk_starts():
    starts, s0 = [], 0
    for L in CHUNKS:
        starts.append(s0)
        s0 += L
    assert s0 == NSB
    return starts


def _build_program():
    nc = bacc.Bacc("TRN2", target_bir_lowering=False, debug=False)
    # input cols per chunk: [re: L*SBC][im: L*SBC], chunk-major
    sig = nc.dram_tensor("sig", [BL, NSB * 2 * SBC], F16,
                         kind="ExternalInput").ap()
    wd = nc.dram_tensor("w", [BL, 3 * BL], F16, kind="ExternalInput").ap()
    yd = nc.dram_tensor("y", [BL, NSB * SBC], F16,
                        kind="ExternalOutput").ap()

    starts = _chunk_starts()
    base = [2 * SBC * s for s in starts]
    NCH = len(CHUNKS)

    with tile.TileContext(nc) as tc, ExitStack() as ctx:
        wpool = ctx.enter_context(tc.tile_pool(name="w", bufs=1))
        wsb = wpool.tile([BL, 3 * BL], F16, tag="w", name="w_sb")
        nc.sync.dma_start(wsb[:], wd)
        w = {"l0": wsb[:, 0:BL], "t0": wsb[:, BL:2 * BL],
             "t1": wsb[:, 2 * BL:3 * BL]}

        sigpool = ctx.enter_context(tc.tile_pool(name="sig", bufs=1))
        p2pool = ctx.enter_context(tc.tile_pool(name="p2", bufs=2))
        xhpool = ctx.enter_context(tc.tile_pool(name="xh", bufs=3))
        yspool = ctx.enter_context(tc.tile_pool(name="ys", bufs=2))
        pspool = ctx.enter_context(tc.tile_pool(name="ps", bufs=2,
                                                space="PSUM"))

        sig_sb = sigpool.tile([BL, NSB * 2 * SBC], F16, tag="sig",
                              name="sig_sb")
        # all input DMAs issued up-front: back-to-back on the Sync ring
        for c, L in enumerate(CHUNKS):
            sp = slice(base[c], base[c] + 2 * L * SBC)
            nc.sync.dma_start(sig_sb[:, sp], sig[:, sp])

        def mm(ps_ap, wt, rhs_ap, start, stop):
            nc.tensor.matmul(ps_ap, w[wt], rhs_ap, start=start, stop=stop,
                             skip_group_check=True)

        MAXW = 2 * SBC                      # widest chunk (L=2) in cols

        pend = []                           # drain entries, FIFO by chunk
        store_q = []                        # [(ys_ap, dram col slice)]
        prev_xh = None
        for c, L in enumerate(CHUNKS):
            W2 = L * SBC
            p2 = p2pool.tile([BL, 2 * MAXW], F16, tag="p2",
                             name="p2_%d" % c)

            xh = xhpool.tile([BL, MAXW], F16, tag="xh")
            ys = yspool.tile([BL, MAXW], F16, tag="ys")

            # GPS is kept OFF the critical chain (stores only): squares
            # split DVE 3/4 (2x tensor_tensor, 0.59 ns/col measured)
            # and ACT 1/4 (the last superbatch's im half, so each add
            # depends only on work finishing early in its slot)
            nc.scalar.activation(p2[:, 3 * W2 // 2:2 * W2],
                                 sig_sb[:, base[c] + 3 * W2 // 2:
                                        base[c] + 2 * W2],
                                 mybir.ActivationFunctionType.Square)
            nc.vector.tensor_mul(p2[:, 0:3 * W2 // 2],
                                 sig_sb[:, base[c]:base[c] + 3 * W2 // 2],
                                 sig_sb[:, base[c]:base[c] + 3 * W2 // 2])
            for l in range(L):
                bs = l * SBC
                nc.vector.tensor_add(xh[:, bs:bs + SBC],
                                     p2[:, bs:bs + SBC],
                                     p2[:, W2 + bs:W2 + bs + SBC])

            # matmuls: one [BL, 8C] PSUM tile per superbatch, wide rhs
            for l in range(L):
                s = starts[c] + l
                bs = l * SBC

                def b(i, n=1):
                    return xh[:, bs + i * C:bs + (i + n) * C]

                ps = pspool.tile([BL, 8 * C], F32, tag="ps",
                                 name="ps_%d" % s)
                if s == 0:
                    # exact-init operator L0 for block 0, no cross term
                    mm(ps[:, 0:C], "l0", b(0), True, True)
                    mm(ps[:, C:2 * C], "t0", b(1), True, False)
                    mm(ps[:, C:2 * C], "t1", b(0), False, True)
                elif l == 0:
                    # cross-chunk T1 term: bank 0's group is opened by
                    # the 2C t0, then t1 lands in two pieces (prev
                    # chunk's last block + own block 0) before closing
                    pxh, pl = prev_xh
                    prevC = pxh[:, pl * SBC - C:pl * SBC]
                    mm(ps[:, 0:2 * C], "t0", b(0, 2), True, False)
                    mm(ps[:, 0:C], "t1", prevC, False, False)
                    mm(ps[:, C:2 * C], "t1", b(0), False, True)
                else:
                    mm(ps[:, 0:2 * C], "t0", b(0, 2), True, False)
                    mm(ps[:, 0:2 * C], "t1", b(-1, 2), False, True)
                # banks 1..3: all t0 opens first, then all t1 closes --
                # one weight switch instead of one per matmul
                for r in range(1, 4):
                    mm(ps[:, 2 * r * C:(2 * r + 2) * C], "t0",
                       b(2 * r, 2), True, False)
                for r in range(1, 4):
                    mm(ps[:, 2 * r * C:(2 * r + 2) * C], "t1",
                       b(2 * r - 1, 2), False, True)
                pend.append((s, l, ps, ys))

            # drains + store for chunk c-1, issued after this chunk's
            # matmuls; in the DMA-paced regime the PE wait is absorbed
            if c >= 1:
                Lp = CHUNKS[c - 1]
                ents, pend = pend[:Lp], pend[Lp:]
                for (ds, dl, dps, dys) in ents:
                    nc.scalar.activation(
                        dys[:, dl * SBC:(dl + 1) * SBC], dps[:],
                        mybir.ActivationFunctionType.Copy)
                ys_ap, cols = store_q.pop(0)
                nc.gpsimd.dma_start(yd[:, cols], ys_ap)

            prev_xh = (xh, L)
            store_q.append((ys[:, 0:W2],
                            slice(starts[c] * SBC, (starts[c] + L) * SBC)))

        # tail: drain + store the final chunk, split across both DMA
        # paths for latency
        Lp = CHUNKS[NCH - 1]
        ents, pend = pend[:Lp], pend[Lp:]
        for (ds, dl, dps, dys) in ents:
            nc.scalar.activation(dys[:, dl * SBC:(dl + 1) * SBC],
                                 dps[:],
                                 mybir.ActivationFunctionType.Copy)
        ys_ap, cols = store_q.pop(0)
        n = cols.stop - cols.start
        nc.sync.dma_start(yd[:, cols.start:cols.start + n // 2],
                          ys_ap[:, 0:n // 2])
        nc.gpsimd.dma_start(yd[:, cols.start + n // 2:cols.stop],
                            ys_ap[:, n // 2:n])
        assert not pend and not store_q

    nc.compile()
    return nc


def kernel(signal, b, a):
    global LAST_RESULTS
    signal = np.asarray(signal)
    assert signal.shape == (2, T, B), signal.shape

    wmat = _build_mats(np.asarray(b), np.asarray(a))

    if "prog" not in _program_cache:
        _program_cache["prog"] = _build_program()
    nc = _program_cache["prog"]

    starts = _chunk_starts()
    # pack to per-core chunk-major fp16 layout:
    # [core, p, chunk{ re[l,b,ch] | im[l,b,ch] }]
    x = signal.reshape(2, NSB, SBW, BL, NCORES, C)
    parts = []
    for c, L in enumerate(CHUNKS):
        xs = x[:, starts[c]:starts[c] + L]        # [2, L, SBW, BL, 8, C]
        parts.append(xs.transpose(4, 3, 0, 1, 2, 5).reshape(
            NCORES, BL, 2 * L * SBW * C))
    pk = np.ascontiguousarray(np.concatenate(parts, axis=2),
                              dtype=np.float16)   # [8, BL, NSB*2*SBC]

    in_maps = [{"sig": pk[c], "w": wmat} for c in range(NCORES)]

    res = run_bass_kernel_spmd(nc, in_maps, core_ids=list(range(NCORES)),
                               trace=TRACE)
    LAST_RESULTS = res

    out = np.empty((T, B), np.float32)
    for c in range(NCORES):
        yc = np.asarray(res.results[c]["y"])      # [BL, NSB*SBC]
        yc = yc.reshape(BL, NSB, SBW, C).transpose(1, 2, 0, 3)
        out[:, c * C:(c + 1) * C] = yc.reshape(T, C).astype(np.float32)
    return out


# revision 21
# speedup vs baseline: 1.1718x; 1.0143x over previous
"""Trainium2 Bass kernel for nn_LowpassDetector.

Computes: power = re^2 + im^2, 5-tap FIR (b), order-4 IIR recurrence (a)
along time, for signal [2, T=16384, B=2048] -> y [T, B].

Strategy: the FIR+IIR cascade is LTI with all poles at radius <= 0.758,
so the combined impulse response decays below 1e-15 within 128 taps.
The whole filter is therefore exactly (to fp32) a block-Toeplitz matmul:
  y_blk[b] = T0 @ x_blk[b] + T1 @ x_blk[b-1]     (b >= 1)
  y_blk[0] = L0 @ x_blk[0]
with L0 the exact 128x128 operator of the reference recurrence
(including its "first 5 samples pass through" initial condition), built
on the host in float64. Channels (2048) are sharded 256 per core across
8 cores; time blocks of 128 map to the TensorEngine contraction dim.

v6 design (from v5's ~91-98 us; trace-driven):
- All I/O fp16 (fp8-e3m4/e4m3 input was simulated on host and fails the
  2e-2 max-rel budget at 3-6.6e-2: the metric is tail-dominated and
  fp8's coarse ulp at large |x| survives the filter). 25.3 MB/core at
  the measured ~420 GB/s sustained = ~60 us hard DMA floor.
- v5's trace: ~7 us fixed preamble, 16 serial input-DMA issues at
  ~0.6 us each, a 23 us tail at 1/3 rate where stores trailed per-SB
  compute, and ~8 us fixed teardown (full semaphore-file clear). v6:
  * The whole input (128 KB/partition) is SBUF-resident: 9 chunk DMAs
    (7x2-SB + 2x1-SB) all issued up-front on the Sync HWDGE queue, so
    input streams back-to-back from ~7.6 us with no further issue
    dependencies. Final chunks are small to shorten the drain tail.
  * Engine split by measured per-column cost (DVE 2x fp16 tensor_tensor
    0.52 ns/col, ACT 0.83, GPS ~2.1): DVE does re^2 (non-in-place, 2x)
    and the power add (2x) chunk-wide; ACT squares 2560/4096 im cols
    and drains 3/4 of each PSUM tile (it sits closest to PSUM); GPS
    squares the other 1536 im cols and issues the lag-2 chunk stores
    on its SWDGE ring. ~5.5-6 us of engine time per 2-SB chunk, under
    the ~6.5 us/chunk global engine budget, so stores are produced
    fast enough to keep the SDMA engines fed to the end.
  * Drains are issued one chunk late (after the next chunk's forward
    elementwise ops) so their PE waits are free; stores lag two chunks.
  * Each chunk's xh has a C-wide margin holding the previous chunk's
    last block (one 256-col copy per chunk) so every matmul rhs --
    including the cross-superbatch T1 operand -- is one contiguous AP.
- PSUM rules (kept from v5, learned the hard way): a matmul output
  region must not straddle a 2 KB bank boundary, and each half-bank
  holds exactly one accumulation group, opened once and closed once.
"""

import sys
from contextlib import ExitStack

import numpy as np

for _p in ("/opt/trn_rl_repo",):
    if _p not in sys.path:
        sys.path.insert(0, _p)

import concourse.bass as bass  # noqa: E402
import concourse.tile as tile  # noqa: E402
from concourse import bacc, mybir  # noqa: E402
from concourse.bass_utils import run_bass_kernel_spmd  # noqa: E402

T, B, NCORES = 16384, 2048, 8
BL = 128                # time-block size (= PE contraction dim)
NB = T // BL            # 128 time blocks
C = B // NCORES         # 256 channels per core
SBW = 8                 # time blocks per superbatch
NSB = NB // SBW         # 16 superbatches
CHUNKS = (1, 2, 2, 2, 2, 2, 2, 1, 1, 1)   # superbatches per chunk
SBC = SBW * C           # 2048 columns per superbatch (one block-row)
F32 = mybir.dt.float32
F16 = mybir.dt.float16

TRACE = False           # set by test harness for NTFF profiling
LAST_RESULTS = None     # BassKernelResults of the last run (for profiling)

_program_cache = {}


def _reference_operator(bb, aa, n):
    """Exact linear operator of the reference filter on n samples (float64).

    Columns are responses to basis vectors; replicates the reference
    semantics: xf = zero-padded cross-correlation with b, first 5 outputs
    pass through, recurrence y[t] = xf[t] - sum_j a_j y[t-j] from t=5.
    """
    x = np.eye(n)
    xp = np.concatenate([np.zeros((4, n)), x], 0)
    xf = sum(bb[k] * xp[k:k + n] for k in range(5))
    y = xf.copy()
    at = aa[:4]
    for t in range(5, n):
        y[t] = xf[t] - (at[0] * y[t - 4] + at[1] * y[t - 3]
                        + at[2] * y[t - 2] + at[3] * y[t - 1])
    return y


def _build_mats(b32, a32):
    """Returns [BL, 3*BL] fp16: the three lhsT operands packed so the
    weights load with a single contiguous DMA (768 B per partition)."""
    bb = np.asarray(b32, np.float64)
    aa = np.asarray(a32, np.float64)
    M = _reference_operator(bb, aa, 3 * BL)
    L0 = M[0:BL, 0:BL]
    T0 = M[2 * BL:3 * BL, 2 * BL:3 * BL]
    T1 = M[2 * BL:3 * BL, BL:2 * BL]
    # truncation + init-transient leakage must be below fp32 noise
    leak = np.abs(M[2 * BL:3 * BL, 0:BL]).max()
    dev = max(np.abs(M[BL:2 * BL, BL:2 * BL] - T0).max(),
              np.abs(M[BL:2 * BL, 0:BL] - T1).max())
    assert leak < 1e-9 and dev < 1e-9, (leak, dev)

    w = np.empty((BL, 3 * BL), np.float16)
    for j, W in enumerate((L0, T0, T1)):
        w[:, j * BL:(j + 1) * BL] = W.T.astype(np.float16)  # lhsT = W.T
    return np.ascontiguousarray(w)


def _chunk_starts():
    starts, s0 = [], 0
    for L in CHUNKS:
        starts.append(s0)
        s0 += L
    assert s0 == NSB
    return starts


def _build_program():
    nc = bacc.Bacc("TRN2", target_bir_lowering=False, debug=False)
    # input cols per chunk: [re: L*SBC][im: L*SBC], chunk-major
    sig = nc.dram_tensor("sig", [BL, NSB * 2 * SBC], F16,
                         kind="ExternalInput").ap()
    wd = nc.dram_tensor("w", [BL, 3 * BL], F16, kind="ExternalInput").ap()
    yd = nc.dram_tensor("y", [BL, NSB * SBC], F16,
                        kind="ExternalOutput").ap()

    starts = _chunk_starts()
    base = [2 * SBC * s for s in starts]
    NCH = len(CHUNKS)

    with tile.TileContext(nc) as tc, ExitStack() as ctx:
        wpool = ctx.enter_context(tc.tile_pool(name="w", bufs=1))
        wsb = wpool.tile([BL, 3 * BL], F16, tag="w", name="w_sb")
        nc.sync.dma_start(wsb[:], wd)
        w = {"l0": wsb[:, 0:BL], "t0": wsb[:, BL:2 * BL],
             "t1": wsb[:, 2 * BL:3 * BL]}

        sigpool = ctx.enter_context(tc.tile_pool(name="sig", bufs=1))
        p2pool = ctx.enter_context(tc.tile_pool(name="p2", bufs=2))
        xhpool = ctx.enter_context(tc.tile_pool(name="xh", bufs=3))
        pspool = ctx.enter_context(tc.tile_pool(name="ps", bufs=2,
                                                space="PSUM"))

        sig_sb = sigpool.tile([BL, NSB * 2 * SBC], F16, tag="sig",
                              name="sig_sb")
        # all input DMAs issued up-front: back-to-back on the Sync ring
        for c, L in enumerate(CHUNKS):
            sp = slice(base[c], base[c] + 2 * L * SBC)
            nc.sync.dma_start(sig_sb[:, sp], sig[:, sp])

        def mm(ps_ap, wt, rhs_ap, start, stop):
            nc.tensor.matmul(ps_ap, w[wt], rhs_ap, start=start, stop=stop,
                             skip_group_check=True)

        MAXW = 2 * SBC                      # widest chunk (L=2) in cols

        pend = []                           # drain entries, FIFO by chunk
        store_q = []                        # [(ys_ap, dram col slice)]
        prev_xh = None
        for c, L in enumerate(CHUNKS):
            W2 = L * SBC
            p2 = p2pool.tile([BL, 2 * MAXW], F16, tag="p2",
                             name="p2_%d" % c)

            xh = xhpool.tile([BL, MAXW], F16, tag="xh")

            # GPS is kept OFF the critical chain (stores only): squares
            # split DVE 3/4 (2x tensor_tensor, 0.59 ns/col measured)
            # and ACT 1/4 (the last superbatch's im half, so each add
            # depends only on work finishing early in its slot)
            nc.scalar.activation(p2[:, 3 * W2 // 2:2 * W2],
                                 sig_sb[:, base[c] + 3 * W2 // 2:
                                        base[c] + 2 * W2],
                                 mybir.ActivationFunctionType.Square)
            nc.vector.tensor_mul(p2[:, 0:3 * W2 // 2],
                                 sig_sb[:, base[c]:base[c] + 3 * W2 // 2],
                                 sig_sb[:, base[c]:base[c] + 3 * W2 // 2])
            for l in range(L):
                bs = l * SBC
                nc.vector.tensor_add(xh[:, bs:bs + SBC],
                                     p2[:, bs:bs + SBC],
                                     p2[:, W2 + bs:W2 + bs + SBC])

            # matmuls: one [BL, 8C] PSUM tile per superbatch, wide rhs
            for l in range(L):
                s = starts[c] + l
                bs = l * SBC

                def b(i, n=1):
                    return xh[:, bs + i * C:bs + (i + n) * C]

                ps = pspool.tile([BL, 8 * C], F32, tag="ps",
                                 name="ps_%d" % s)
                if s == 0:
                    # exact-init operator L0 for block 0, no cross term
                    mm(ps[:, 0:C], "l0", b(0), True, True)
                    mm(ps[:, C:2 * C], "t0", b(1), True, False)
                    mm(ps[:, C:2 * C], "t1", b(0), False, True)
                elif l == 0:
                    # cross-chunk T1 term: bank 0's group is opened by
                    # the 2C t0, then t1 lands in two pieces (prev
                    # chunk's last block + own block 0) before closing
                    pxh, pl = prev_xh
                    prevC = pxh[:, pl * SBC - C:pl * SBC]
                    mm(ps[:, 0:2 * C], "t0", b(0, 2), True, False)
                    mm(ps[:, 0:C], "t1", prevC, False, False)
                    mm(ps[:, C:2 * C], "t1", b(0), False, True)
                else:
                    mm(ps[:, 0:2 * C], "t0", b(0, 2), True, False)
                    mm(ps[:, 0:2 * C], "t1", b(-1, 2), False, True)
                # banks 1..3: all t0 opens first, then all t1 closes --
                # one weight switch instead of one per matmul
                for r in range(1, 4):
                    mm(ps[:, 2 * r * C:(2 * r + 2) * C], "t0",
                       b(2 * r, 2), True, False)
                for r in range(1, 4):
                    mm(ps[:, 2 * r * C:(2 * r + 2) * C], "t1",
                       b(2 * r - 1, 2), False, True)
                pend.append((s, l, ps, c))

            # drains for chunk c-1 write into its own DEAD input region
            # (consumed by slot c-1's squares), then the store goes on
            # the Sync HWDGE ring BEHIND the input DMAs: ring FIFO =
            # strict input priority, and the dead regions are an
            # unlimited store backlog
            if c >= 1:
                Lp = CHUNKS[c - 1]
                ents, pend = pend[:Lp], pend[Lp:]
                for (ds, dl, dps, dc) in ents:
                    nc.scalar.activation(
                        sig_sb[:, base[dc] + dl * SBC:
                               base[dc] + (dl + 1) * SBC], dps[:],
                        mybir.ActivationFunctionType.Copy)
                src_ap, cols = store_q.pop(0)
                nc.sync.dma_start(yd[:, cols], src_ap)

            prev_xh = (xh, L)
            store_q.append((sig_sb[:, base[c]:base[c] + W2],
                            slice(starts[c] * SBC, (starts[c] + L) * SBC)))

        # tail: drain + store the final chunk; the GPS half rides the
        # empty SWDGE ring so it starts before the Sync ring drains
        Lp = CHUNKS[NCH - 1]
        ents, pend = pend[:Lp], pend[Lp:]
        for (ds, dl, dps, dc) in ents:
            nc.scalar.activation(sig_sb[:, base[dc] + dl * SBC:
                                        base[dc] + (dl + 1) * SBC],
                                 dps[:],
                                 mybir.ActivationFunctionType.Copy)
        src_ap, cols = store_q.pop(0)
        n = cols.stop - cols.start
        nc.gpsimd.dma_start(yd[:, cols.start:cols.start + n // 2],
                            src_ap[:, 0:n // 2])
        nc.sync.dma_start(yd[:, cols.start + n // 2:cols.stop],
                          src_ap[:, n // 2:n])
        assert not pend and not store_q

    nc.compile()
    return nc


def kernel(signal, b, a):
    global LAST_RESULTS
    signal = np.asarray(signal)
    assert signal.shape == (2, T, B), signal.shape

    wmat = _build_mats(np.asarray(b), np.asarray(a))

    if "prog" not in _program_cache:
        _program_cache["prog"] = _build_program()
    nc = _program_cache["prog"]

    starts = _chunk_starts()
    # pack to per-core chunk-major fp16 layout:
    # [core, p, chunk{ re[l,b,ch] | im[l,b,ch] }]
    x = signal.reshape(2, NSB, SBW, BL, NCORES, C)
    parts = []
    for c, L in enumerate(CHUNKS):
        xs = x[:, starts[c]:starts[c] + L]        # [2, L, SBW, BL, 8, C]
        parts.append(xs.transpose(4, 3, 0, 1, 2, 5).reshape(
            NCORES, BL, 2 * L * SBW * C))
    pk = np.ascontiguousarray(np.concatenate(parts, axis=2),
                              dtype=np.float16)   # [8, BL, NSB*2*SBC]

    in_maps = [{"sig": pk[c], "w": wmat} for c in range(NCORES)]

    res = run_bass_kernel_spmd(nc, in_maps, core_ids=list(range(NCORES)),
                               trace=TRACE)
    LAST_RESULTS = res

    out = np.empty((T, B), np.float32)
    for c in range(NCORES):
        yc = np.asarray(res.results[c]["y"])      # [BL, NSB*SBC]
        yc = yc.reshape(BL, NSB, SBW, C).transpose(1, 2, 0, 3)
        out[:, c * C:(c + 1) * C] = yc.reshape(T, C).astype(np.float32)
    return out
